# revision 1
# baseline (speedup 1.0000x reference)
"""Trainium2 Bass kernel for GraphSpectralFilterLayer.

Reference computation:
    h = x @ W.T                                  [4096, 128]
    mean = attention.mean()                      (global scalar)
    a = where(att > mean, att, -9e15); LeakyReLU(0.2); softmax(dim=0)
    a = where(drop_mask, a / 0.4, 0)
    out = (a @ h) reshaped (4,4096,128)->(4096, 512)

Exact simplifications (att ~ U[0,1), mean > 0):
    - kept values are positive so LeakyReLU is identity on them; dropped
      values give exp(0.2 * -9e15) == 0 exactly in f32. Hence
      v = exp(att) * (att > mean), softmax = v / colsum(v)  (shift-free
      exp is safe: att in [0,1)).
    - softmax denominator does NOT include the dropout mask.
    - (v / colsum) @ h == v @ (h * (1/(0.4*colsum))[:, None]) -- fold
      normalization + dropout scale into the tiny h matrix.

Sharding: rows of the [16384, 4096] attention matrix across 8 cores
(2048 rows each). softmax(dim=0) needs only a [128, 32] AllReduce of
column sums; the global mean is a [1, 1] AllReduce. attention/mask are
passed host-transposed so tiles land in SBUF with the contraction dim
(j, graph nodes) on partitions -- no on-chip transposes. Output is
produced transposed [128, 2048] per core, un-transposed on host.
"""

import sys

sys.path.insert(0, "/opt/trn_rl_repo")

import numpy as np

from concourse import bass, bacc, tile, mybir
from concourse.bass_utils import run_bass_kernel_spmd

N = 4096          # graph nodes (columns of attention)
CN = 16384        # C * N rows of attention
OUT_F = 128
IN_F = 512
N_CORES = 8
ROWS = CN // N_CORES          # 2048 local attention rows (i)
JT = N // 128                 # 32 j-tiles
JC = 8                        # j-tiles of v cached in SBUF (of 32)
NCN = float(CN) * float(N)    # mean divisor

F32 = mybir.dt.float32
U8 = mybir.dt.uint8
AX = mybir.AxisListType
OP = mybir.AluOpType
AF = mybir.ActivationFunctionType


def _build():
    nc = bacc.Bacc("TRN2", target_bir_lowering=False, debug=False,
                   num_devices=N_CORES)

    attT = nc.dram_tensor("attT", [N, ROWS], F32, kind="ExternalInput")
    maskT = nc.dram_tensor("maskT", [N, ROWS], U8, kind="ExternalInput")
    xT = nc.dram_tensor("xT", [IN_F, N], F32, kind="ExternalInput")
    wT = nc.dram_tensor("wT", [IN_F, OUT_F], F32, kind="ExternalInput")
    outT = nc.dram_tensor("outT", [OUT_F, ROWS], F32, kind="ExternalOutput")

    with tile.TileContext(nc) as tc:
        with tc.tile_pool(name="persist", bufs=1) as persist, \
             tc.tile_pool(name="dram", bufs=1, space="DRAM") as dram:
            # ---- persistent SBUF state ----
            vcache = persist.tile([128, JC * ROWS], F32)   # cached v tiles
            h = persist.tile([128, JT * OUT_F], F32)       # h, per j-tile
            acc = persist.tile([128, JT], F32)             # mean partials
            cs = persist.tile([128, JT], F32)              # colsum partials
            ones_ff = persist.tile([128, 128], F32)
            tot = persist.tile([128, 1], F32)
            gsum = persist.tile([128, 1], F32)
            mean_bc = persist.tile([128, 1], F32)
            csum = persist.tile([128, JT], F32)
            rcs2 = persist.tile([128, JT], F32)
            nc.vector.memset(ones_ff[:, :], 1.0)

            # collective bounce buffers (DRAM, non-I/O)
            cc_mean_in = dram.tile([128, 1], F32)
            cc_mean_out = dram.tile([128, 1], F32)
            cc_cs_in = dram.tile([128, JT], F32)
            cc_cs_out = dram.tile([128, JT], F32)

            # ---- h = x @ W.T  (from host-transposed xT, wT) ----
            with tc.tile_pool(name="xw", bufs=1) as xw, \
                 tc.tile_pool(name="hps", bufs=2, space="PSUM") as hps:
                wt_t = xw.tile([128, 4 * OUT_F], F32, tag="wt")
                xt_ts = []
                for ct in range(4):
                    nc.sync.dma_start(
                        out=wt_t[:, ct * OUT_F:(ct + 1) * OUT_F],
                        in_=wT[ct * 128:(ct + 1) * 128, :])
                    xt_t = xw.tile([128, N], F32, tag=f"xt{ct}")
                    nc.sync.dma_start(out=xt_t[:, :],
                                      in_=xT[ct * 128:(ct + 1) * 128, :])
                    xt_ts.append(xt_t)
                for jt in range(JT):
                    ps = hps.tile([128, OUT_F], F32, tag="hps")
                    for ct in range(4):
                        nc.tensor.matmul(
                            ps[:, :],
                            lhsT=xt_ts[ct][:, jt * 128:(jt + 1) * 128],
                            rhs=wt_t[:, ct * OUT_F:(ct + 1) * OUT_F],
                            start=(ct == 0), stop=(ct == 3))
                    nc.scalar.copy(h[:, jt * OUT_F:(jt + 1) * OUT_F],
                                   ps[:, :])

            # ---- P1: global mean ----
            with tc.tile_pool(name="p1", bufs=4) as p1:
                for jt in range(JT):
                    a_t = p1.tile([128, ROWS], F32, tag="a1")
                    nc.sync.dma_start(out=a_t[:, :],
                                      in_=attT[jt * 128:(jt + 1) * 128, :])
                    nc.vector.tensor_reduce(out=acc[:, jt:jt + 1],
                                            in_=a_t[:, :], axis=AX.X,
                                            op=OP.add)
            with tc.tile_pool(name="p1b", bufs=1, space="PSUM") as p1b:
                nc.vector.tensor_reduce(out=tot[:, :], in_=acc[:, :],
                                        axis=AX.X, op=OP.add)
                nc.sync.dma_start(out=cc_mean_in[:, :], in_=tot[:, :])
                nc.gpsimd.collective_compute(
                    "AllReduce", OP.add,
                    replica_groups=[list(range(N_CORES))],
                    ins=[cc_mean_in[:, :].opt()],
                    outs=[cc_mean_out[:, :].opt()])
                nc.gpsimd.dma_start(out=gsum[:, :], in_=cc_mean_out[:, :])
                ps_bc = p1b.tile([128, 1], F32, tag="bc")
                nc.tensor.matmul(ps_bc[:, :], lhsT=ones_ff[:, :],
                                 rhs=gsum[:, :], start=True, stop=True)
                nc.vector.tensor_scalar(out=mean_bc[:, :], in0=ps_bc[:, :],
                                        scalar1=1.0 / NCN, scalar2=None,
                                        op0=OP.mult)

            # ---- P2: v = exp(att) * (att > mean); column sums ----
            with tc.tile_pool(name="p2", bufs=3) as p2:
                for jt in range(JT):
                    a_t = p2.tile([128, ROWS], F32, tag="a2")
                    nc.sync.dma_start(out=a_t[:, :],
                                      in_=attT[jt * 128:(jt + 1) * 128, :])
                    e_t = p2.tile([128, ROWS], F32, tag="e2")
                    nc.scalar.activation(e_t[:, :], a_t[:, :], AF.Exp)
                    g_t = p2.tile([128, ROWS], F32, tag="g2")
                    nc.vector.tensor_scalar(out=g_t[:, :], in0=a_t[:, :],
                                            scalar1=mean_bc[:, 0:1],
                                            scalar2=None, op0=OP.is_gt)
                    if jt < JC:
                        v_dst = vcache[:, jt * ROWS:(jt + 1) * ROWS]
                    else:
                        v_t = p2.tile([128, ROWS], F32, tag="v2")
                        v_dst = v_t[:, :]
                    nc.vector.tensor_tensor(out=v_dst, in0=g_t[:, :],
                                            in1=e_t[:, :], op=OP.mult)
                    nc.vector.tensor_reduce(out=cs[:, jt:jt + 1], in_=v_dst,
                                            axis=AX.X, op=OP.add)

            # ---- colsum AllReduce; fold 1/(0.4*colsum) into h ----
            nc.sync.dma_start(out=cc_cs_in[:, :], in_=cs[:, :])
            nc.gpsimd.collective_compute(
                "AllReduce", OP.add,
                replica_groups=[list(range(N_CORES))],
                ins=[cc_cs_in[:, :].opt()],
                outs=[cc_cs_out[:, :].opt()])
            nc.sync.dma_start(out=csum[:, :], in_=cc_cs_out[:, :])
            nc.vector.tensor_scalar(out=rcs2[:, :], in0=csum[:, :],
                                    scalar1=0.4, scalar2=None, op0=OP.mult)
            nc.vector.reciprocal(rcs2[:, :], rcs2[:, :])
            for jt in range(JT):
                nc.vector.tensor_scalar(
                    out=h[:, jt * OUT_F:(jt + 1) * OUT_F],
                    in0=h[:, jt * OUT_F:(jt + 1) * OUT_F],
                    scalar1=rcs2[:, jt:jt + 1], scalar2=None, op0=OP.mult)

            # ---- P3: vm = v * mask; outT[f, i] += h_s[jt].T @ vm ----
            with tc.tile_pool(name="p3", bufs=3) as p3, \
                 tc.tile_pool(name="p3r", bufs=1) as p3r, \
                 tc.tile_pool(name="ops", bufs=1, space="PSUM") as ops:
                ps_o = []
                for ic in range(4):
                    ps_oc = ops.tile([128, 512], F32, tag=f"o{ic}")
                    ps_o.append(ps_oc)
                for jt in range(JT):
                    if jt < JC:
                        v_src = vcache[:, jt * ROWS:(jt + 1) * ROWS]
                    else:
                        a_t = p3r.tile([128, ROWS], F32, tag="a3")
                        nc.sync.dma_start(
                            out=a_t[:, :],
                            in_=attT[jt * 128:(jt + 1) * 128, :])
                        e_t = p3r.tile([128, ROWS], F32, tag="e3")
                        nc.scalar.activation(e_t[:, :], a_t[:, :], AF.Exp)
                        g_t = p3r.tile([128, ROWS], F32, tag="g3")
                        nc.vector.tensor_scalar(out=g_t[:, :], in0=a_t[:, :],
                                                scalar1=mean_bc[:, 0:1],
                                                scalar2=None, op0=OP.is_gt)
                        v_t = p3r.tile([128, ROWS], F32, tag="v3")
                        nc.vector.tensor_tensor(out=v_t[:, :], in0=g_t[:, :],
                                                in1=e_t[:, :], op=OP.mult)
                        v_src = v_t[:, :]
                    m_t = p3.tile([128, ROWS], U8, tag="m3")
                    nc.sync.dma_start(out=m_t[:, :],
                                      in_=maskT[jt * 128:(jt + 1) * 128, :])
                    mf_t = p3.tile([128, ROWS], F32, tag="mf3")
                    nc.gpsimd.tensor_copy(mf_t[:, :], m_t[:, :])
                    vm_t = p3.tile([128, ROWS], F32, tag="vm3")
                    nc.gpsimd.tensor_tensor(out=vm_t[:, :], in0=v_src,
                                            in1=mf_t[:, :], op=OP.mult)
                    for ic in range(4):
                        nc.tensor.matmul(
                            ps_o[ic][:, :],
                            lhsT=h[:, jt * OUT_F:(jt + 1) * OUT_F],
                            rhs=vm_t[:, ic * 512:(ic + 1) * 512],
                            start=(jt == 0), stop=(jt == JT - 1))
                for ic in range(4):
                    o_t = p3.tile([128, 512], F32, tag="osb")
                    nc.scalar.copy(o_t[:, :], ps_o[ic][:, :])
                    nc.sync.dma_start(out=outT[:, ic * 512:(ic + 1) * 512],
                                      in_=o_t[:, :])
    nc.compile()
    return nc


def kernel(x, attention, W, drop_mask):
    attT = np.ascontiguousarray(attention.T)           # [4096, 16384] f32
    maskT = np.ascontiguousarray(
        drop_mask.astype(np.uint8, copy=False).T)      # [4096, 16384] u8
    xT = np.ascontiguousarray(x.T)                     # [512, 4096]
    wT = np.ascontiguousarray(W.T)                     # [512, 128]

    nc = _build()
    in_maps = []
    for c in range(N_CORES):
        sl = slice(c * ROWS, (c + 1) * ROWS)
        in_maps.append({
            "attT": np.ascontiguousarray(attT[:, sl]),
            "maskT": np.ascontiguousarray(maskT[:, sl]),
            "xT": xT,
            "wT": wT,
        })
    res = run_bass_kernel_spmd(nc, in_maps, core_ids=list(range(N_CORES)))
    global LAST_EXEC_NS
    LAST_EXEC_NS = res.exec_time_ns or res.mean_exec_time_ns
    h_prime = np.concatenate(
        [res.results[c]["outT"].T for c in range(N_CORES)], axis=0)
    out = (h_prime.reshape(4, N, OUT_F).transpose(1, 0, 2)
           .reshape(N, 4 * OUT_F))
    return np.ascontiguousarray(out)


if __name__ == "__main__":
    rng = np.random.default_rng(0)
    x = rng.standard_normal((N, IN_F), dtype=np.float32)
    att = rng.random((CN, N), dtype=np.float32)
    W = (rng.standard_normal((OUT_F, IN_F), dtype=np.float32)
         / np.sqrt(IN_F)).astype(np.float32)
    dm = rng.integers(0, 2, size=(CN, N)).astype(bool)
    out = kernel(x=x, attention=att, W=W, drop_mask=dm)
    print("kernel out", out.shape, out.dtype, float(np.abs(out).max()))



# revision 40
# speedup vs baseline: 2.5482x; 2.5482x over previous
"""Trainium2 Bass kernel for GraphSpectralFilterLayer.

Reference computation:
    h = x @ W.T                                  [4096, 128]
    mean = attention.mean()                      (global scalar)
    a = where(att > mean, att, -9e15); LeakyReLU(0.2); softmax(dim=0)
    a = where(drop_mask, a / 0.4, 0)
    out = (a @ h) reshaped (4,4096,128)->(4096, 512)

Exact simplifications (att ~ U[0,1), mean > 0): kept values are positive
so LeakyReLU is identity; dropped values give exp(0.2*-9e15) == 0. Hence
v = exp(att) * (att > mean), softmax = v / colsum(v) (denominator does
NOT include the dropout mask). 1/(0.4*colsum) is folded into the tiny h.

Design (single pass over attention instead of three):
  - Phase 1 streams each att tile once: ACT caches t = att - 0.5 in a
    16 MiB bf16 SBUF cache while DVE accumulates mean partials via
    scalar_tensor_tensor accum_out. Caching the shifted t (not exp) puts
    full bf16 absolute precision at the threshold, so the phase-2 gate
    (t > mean - 0.5) is bit-faithful where it matters; the uniform
    e^0.5 factor cancels between numerator and colsum.
  - Mean finalized with a ones-matmul partition reduce + AllGather +
    local sum (flat AllGather is ~1.9x cheaper than AllReduce here).
  - Phase 2 per tile: ACT computes exp(t) into a small ring; one DVE
    scalar_tensor_tensor produces v = (t > m')*exp(t) in place in the
    cache WITH its column-sum (accum_out); vm = v * mask runs on GpSimd
    for the first 26 tiles (bf16 1/0 mask, host-converted) and DVE for
    the rest. Colsums AllGather in 3 chunks so only the last is exposed;
    h scaling (ACT, per-partition reciprocal colsum) and the bf16 PSUM
    matmuls out[f,i] += h_s[jt].T @ vm[jt] chase each chunk.
  - x/W are host-cast to bf16; h = x @ W.T on PE overlaps the mean
    AllGather window, pre-scaled by 1/0.4.

Sharding: rows of the [16384, 4096] attention matrix across 8 cores
(2048 rows each); attention/mask host-transposed so the contraction dim
(j, graph nodes) lands on partitions. softmax(dim=0) needs only the
[128, 32] colsum exchange + a scalar mean exchange. Output is produced
transposed [128, 2048] per core and un-transposed on host.
"""

import sys

sys.path.insert(0, "/opt/trn_rl_repo")

import os

import numpy as np

from concourse import bass, bacc, tile, mybir
from concourse.bass_utils import run_bass_kernel_spmd

N = 4096          # graph nodes (columns of attention)
CN = 16384        # C * N rows of attention
OUT_F = 128
IN_F = 512
N_CORES = 8
ROWS = CN // N_CORES          # 2048 local attention rows (i)
JT = N // 128                 # 32 j-tiles
NCN = float(CN) * float(N)    # mean divisor

F32 = mybir.dt.float32
BF16 = mybir.dt.bfloat16
U8 = mybir.dt.uint8
U16 = mybir.dt.uint16
AX = mybir.AxisListType
OP = mybir.AluOpType
AF = mybir.ActivationFunctionType



STAGE = int(os.environ.get("K_STAGE", "9"))
CHUNKS = [(0, 10), (10, 20), (20, 32)]
# gate stt runs on DVE only (walrus rejects TensorScalarPtr on Pool);
# the vm = v*mask tensor_tensor ops split: Pool takes the first 20
# tiles (ready earliest), DVE the last 12 after its stt stream.
POOL_TT = frozenset(range(20))
TT_ORDER = list(range(JT))


def _build():
    nc = bacc.Bacc("TRN2", target_bir_lowering=False, debug=False,
                   num_devices=N_CORES)

    attT = nc.dram_tensor("attT", [N, ROWS], F32, kind="ExternalInput")
    maskT = nc.dram_tensor("maskT", [N, ROWS], BF16, kind="ExternalInput")
    xT = nc.dram_tensor("xT", [IN_F, N], BF16, kind="ExternalInput")
    wT = nc.dram_tensor("wT", [IN_F, OUT_F], BF16, kind="ExternalInput")
    outT = nc.dram_tensor("outT", [OUT_F, ROWS], F32, kind="ExternalOutput")

    with tile.TileContext(nc) as tc:
        with tc.tile_pool(name="persist", bufs=1) as persist, \
             tc.tile_pool(name="mring", bufs=4) as mring, \
             tc.tile_pool(name="dram", bufs=1, space="DRAM") as dram:
            # ---- persistent SBUF state ----
            ecache = persist.tile([128, JT * ROWS], BF16)  # e, then vm
            h = persist.tile([128, JT * OUT_F], BF16)      # h per j-tile
            hs = persist.tile([128, JT * OUT_F], BF16)     # scaled h
            acc = persist.tile([128, JT], F32)             # mean partials
            cs = []
            for k, w in enumerate((10, 10, 12)):
                cs_k = persist.tile([128, w], F32, tag=f"cs{k}")
                cs.append(cs_k)
            csg = []
            for k in range(3):
                csg_k = persist.tile([128, N_CORES * 12], F32, tag=f"csg{k}")
                csg.append(csg_k)
            meang = persist.tile([128, N_CORES], F32)      # gathered mean
            ones_ff = persist.tile([128, 128], F32)
            tot = persist.tile([128, 1], F32)
            tot_bc = persist.tile([128, 1], F32)
            em = persist.tile([128, 1], F32)               # exp(mean)
            csum = persist.tile([128, JT], F32)
            rcs = persist.tile([128, JT], F32)
            nc.vector.memset(ones_ff[:, :], 1.0)

            # collective bounce buffers (DRAM, non-I/O)
            ag_mean_in = dram.tile([128, 1], F32)
            ag_mean_out = dram.tile([N_CORES, 128, 1], F32)
            ag_cs_in = []
            ag_cs_out = []
            for k, w in enumerate(c[1] - c[0] for c in CHUNKS):
                ag_cs_in_k = dram.tile([128, w], F32, tag=f"agci{k}")
                ag_cs_out_k = dram.tile([N_CORES, 128, w], F32,
                                        tag=f"agco{k}")
                ag_cs_in.append(ag_cs_in_k)
                ag_cs_out.append(ag_cs_out_k)

            if STAGE < 2:
                nc.compile()
                return nc
            # ---- Phase 1: stream att once; e=exp(att)->cache; mean ----
            m_ts = {}
            ctx2 = tc.tile_pool(name="vscr", bufs=3)
            vscr = ctx2.__enter__()
            with tc.tile_pool(name="p1", bufs=3) as p1:
                for jt in range(JT):
                    a_t = p1.tile([128, ROWS], F32, tag="a1")
                    nc.sync.dma_start(out=a_t[:, :],
                                      in_=attT[jt * 128:(jt + 1) * 128, :])
                    nc.scalar.activation(
                        ecache[:, jt * ROWS:(jt + 1) * ROWS], a_t[:, :],
                        AF.Copy, bias=-0.5)
                    scr = vscr.tile([128, ROWS], BF16, tag="v")
                    nc.vector.scalar_tensor_tensor(
                        out=scr[:, :], in0=a_t[:, :], scalar=-1e30,
                        in1=a_t[:, :], op0=OP.is_gt, op1=OP.mult,
                        accum_out=acc[:, jt:jt + 1])

            def mask_fetch(jt):
                if jt in m_ts or jt >= JT:
                    return
                m_t = mring.tile([128, ROWS], BF16, tag="m")
                nc.sync.dma_start(out=m_t[:, :],
                                  in_=maskT[jt * 128:(jt + 1) * 128, :])
                m_ts[jt] = m_t

            # ---- mean: partition-reduce, AllGather, local sum ----
            with tc.tile_pool(name="mps", bufs=1, space="PSUM") as mps:
                nc.vector.tensor_reduce(out=tot[:, :], in_=acc[:, :],
                                        axis=AX.X, op=OP.add)
                ps_bc = mps.tile([128, 1], F32, tag="bc")
                nc.tensor.matmul(ps_bc[:, :], lhsT=ones_ff[:, :],
                                 rhs=tot[:, :], start=True, stop=True)
                nc.scalar.copy(tot_bc[:, :], ps_bc[:, :])
            nc.sync.dma_start(out=ag_mean_in[:, :], in_=tot_bc[:, :])
            nc.gpsimd.collective_compute(
                "AllGather", OP.bypass,
                replica_groups=[list(range(N_CORES))],
                ins=[ag_mean_in[:, :].opt()],
                outs=[ag_mean_out[:, :, :].opt()])
            # prefetch mask tiles in phase-2 consumption order; fills the
            # mean-AllGather window (issued after the mean bounce so they
            # don't delay it)
            for jt in TT_ORDER[:4]:
                mask_fetch(jt)
            for c in range(N_CORES):
                nc.sync.dma_start(out=meang[:, c:c + 1],
                                  in_=ag_mean_out[c, :, :])
            # tree-sum the 8 gathered copies -> total; em = exp(mean)
            nc.vector.tensor_tensor(out=meang[:, 0:4], in0=meang[:, 0:4],
                                    in1=meang[:, 4:8], op=OP.add)
            nc.vector.tensor_tensor(out=meang[:, 0:2], in0=meang[:, 0:2],
                                    in1=meang[:, 2:4], op=OP.add)
            nc.vector.tensor_tensor(out=meang[:, 0:1], in0=meang[:, 0:1],
                                    in1=meang[:, 1:2], op=OP.add)
            nc.scalar.activation(em[:, :], meang[:, 0:1], AF.Copy,
                                 scale=1.0 / NCN, bias=-0.5)
            # warm the Exp table before phase 2 needs it
            nc.scalar.activation(tot_bc[:, :], em[:, :], AF.Exp)

            # ---- h = x @ W.T  (bf16, from host-transposed xT, wT) ----
            with tc.tile_pool(name="xw", bufs=1) as xw, \
                 tc.tile_pool(name="hps", bufs=2, space="PSUM") as hps:
                wt_t = xw.tile([128, 4 * OUT_F], BF16, tag="wt")
                xt_ts = []
                for ct in range(4):
                    nc.sync.dma_start(
                        out=wt_t[:, ct * OUT_F:(ct + 1) * OUT_F],
                        in_=wT[ct * 128:(ct + 1) * 128, :])
                    xt_t = xw.tile([128, N], BF16, tag=f"xt{ct}")
                    nc.sync.dma_start(out=xt_t[:, :],
                                      in_=xT[ct * 128:(ct + 1) * 128, :])
                    xt_ts.append(xt_t)
                for jt in range(JT):
                    ps = hps.tile([128, OUT_F], F32, tag="hps")
                    for ct in range(4):
                        nc.tensor.matmul(
                            ps[:, :],
                            lhsT=xt_ts[ct][:, jt * 128:(jt + 1) * 128],
                            rhs=wt_t[:, ct * OUT_F:(ct + 1) * OUT_F],
                            start=(ct == 0), stop=(ct == 3))
                    nc.scalar.mul(h[:, jt * OUT_F:(jt + 1) * OUT_F],
                                  ps[:, :], 1.0 / (1.0 - 0.6))



            if STAGE < 3:
                ctx2.__exit__(None, None, None)
                nc.compile()
                return nc

            if STAGE < 4:
                ctx2.__exit__(None, None, None)
                nc.compile()
                return nc
            # ---- Phase 2 + chunked colsum AllGather + matmul ----
            # Streams are laid out so the Pool SEQ meets each collective
            # only after that collective's input is (nearly) ready -- a
            # collective's sem wait head-of-line-blocks everything behind
            # it on the Pool queue.  Chunk 1 is DVE-only so its colsum is
            # ready ~20us into phase 2; matmuls for each chunk run as soon
            # as its scaled h lands, overlapping the remaining gating.
            with tc.tile_pool(name="ops", bufs=1, space="PSUM") as ops:
                ps_o = []
                for ic in range(4):
                    ps_oc = ops.tile([128, 512], F32, tag=f"o{ic}")
                    ps_o.append(ps_oc)

                def e_slice(jt):
                    return ecache[:, jt * ROWS:(jt + 1) * ROWS]

                def cs_slot(jt):
                    for k, (lo, hi) in enumerate(CHUNKS):
                        if lo <= jt < hi:
                            return cs[k][:, jt - lo:jt - lo + 1]

                def stt(jt):
                    e_sl = e_slice(jt)
                    et = vscr.tile([128, ROWS], BF16, tag="v")
                    nc.scalar.activation(et[:, :], e_sl, AF.Exp)
                    nc.vector.scalar_tensor_tensor(
                        out=e_sl, in0=e_sl, scalar=em[:, 0:1],
                        in1=et[:, :], op0=OP.is_gt, op1=OP.mult,
                        accum_out=cs_slot(jt))

                def tt(jt):
                    if jt + 4 < JT:
                        mask_fetch(jt + 4)
                    e_sl = e_slice(jt)
                    eng = nc.gpsimd if jt in POOL_TT else nc.vector
                    eng.tensor_tensor(out=e_sl, in0=e_sl,
                                      in1=m_ts[jt][:, :], op=OP.mult)

                def cs_bounce(k):
                    lo, hi = CHUNKS[k]
                    nc.sync.dma_start(out=ag_cs_in[k][:, :], in_=cs[:, lo:hi])

                def cs_collective(k):
                    nc.gpsimd.collective_compute(
                        "AllGather", OP.bypass,
                        replica_groups=[list(range(N_CORES))],
                        ins=[ag_cs_in[k][:, :].opt()],
                        outs=[ag_cs_out[k][:, :, :].opt()])

                def cs_returns(k):
                    lo, hi = CHUNKS[k]
                    half = hi - lo
                    for c in range(N_CORES):
                        nc.scalar.dma_start(
                            out=csg[k][:, c * half:(c + 1) * half],
                            in_=ag_cs_out[k][c, :, :])

                def cs_trees(k):
                    lo, hi = CHUNKS[k]
                    half = hi - lo
                    sl = slice(lo, hi)
                    nc.vector.tensor_tensor(
                        out=csg[k][:, 0:4 * half], in0=csg[k][:, 0:4 * half],
                        in1=csg[k][:, 4 * half:8 * half], op=OP.add)
                    nc.vector.tensor_tensor(
                        out=csg[k][:, 0:2 * half], in0=csg[k][:, 0:2 * half],
                        in1=csg[k][:, 2 * half:4 * half], op=OP.add)
                    nc.vector.tensor_tensor(
                        out=csum[:, sl], in0=csg[k][:, 0:half],
                        in1=csg[k][:, half:2 * half], op=OP.add)
                    nc.vector.reciprocal(rcs[:, sl], csum[:, sl])

                def scale_and_matmul(jts, last=False):
                    for jt in jts:
                        nc.scalar.mul(
                            hs[:, jt * OUT_F:(jt + 1) * OUT_F],
                            h[:, jt * OUT_F:(jt + 1) * OUT_F],
                            rcs[:, jt:jt + 1])
                        for ic in range(4):
                            nc.tensor.matmul(
                                ps_o[ic][:, :],
                                lhsT=hs[:, jt * OUT_F:(jt + 1) * OUT_F],
                                rhs=ecache[:, jt * ROWS + ic * 512:
                                           jt * ROWS + (ic + 1) * 512],
                                start=(jt == 0), stop=(jt == JT - 1))

                C1, C2, C3 = [list(range(*c)) for c in CHUNKS]

                for jt in C1:
                    stt(jt)
                cs_bounce(0)
                cs_collective(0)
                for jt in C1:
                    tt(jt)            # pool
                for jt in C2:
                    stt(jt)
                cs_bounce(1)
                cs_collective(1)
                for jt in C2:
                    tt(jt)            # pool / dve tail
                cs_finish(0)
                scale_and_matmul(C1)
                for jt in C3:
                    stt(jt)
                cs_bounce(2)
                cs_collective(2)
                for jt in C3:
                    tt(jt)            # dve
                cs_finish(1)
                scale_and_matmul(C2)
                cs_finish(2)
                scale_and_matmul(C3)
                ctx2.__exit__(None, None, None)
                # ---- drain PSUM -> SBUF -> HBM ----
                with tc.tile_pool(name="outp", bufs=2) as outp:
                    for ic in range(4):
                        o_t = outp.tile([128, 512], F32, tag="osb")
                        nc.scalar.copy(o_t[:, :], ps_o[ic][:, :])
                        nc.sync.dma_start(
                            out=outT[:, ic * 512:(ic + 1) * 512],
                            in_=o_t[:, :])
    nc.compile()
    return nc


def kernel(x, attention, W, drop_mask):
    import ml_dtypes
    attT = np.ascontiguousarray(attention.T)           # [4096, 16384] f32
    maskT = np.ascontiguousarray(
        drop_mask.T.astype(ml_dtypes.bfloat16))        # 1.0 / 0.0
    xTb = np.ascontiguousarray(x.T).astype(ml_dtypes.bfloat16)  # [512, 4096]
    wTb = np.ascontiguousarray(W.T).astype(ml_dtypes.bfloat16)  # [512, 128]

    nc = _build()
    in_maps = []
    for c in range(N_CORES):
        sl = slice(c * ROWS, (c + 1) * ROWS)
        in_maps.append({
            "attT": np.ascontiguousarray(attT[:, sl]),
            "maskT": np.ascontiguousarray(maskT[:, sl]),
            "xT": xTb,
            "wT": wTb,
        })
    res = run_bass_kernel_spmd(nc, in_maps, core_ids=list(range(N_CORES)))
    global LAST_EXEC_NS
    LAST_EXEC_NS = res.exec_time_ns or res.mean_exec_time_ns
    h_prime = np.concatenate(
        [res.results[c]["outT"].T for c in range(N_CORES)], axis=0)
    out = (h_prime.reshape(4, N, OUT_F).transpose(1, 0, 2)
           .reshape(N, 4 * OUT_F))
    return np.ascontiguousarray(out)


if __name__ == "__main__":
    rng = np.random.default_rng(0)
    x = rng.standard_normal((N, IN_F), dtype=np.float32)
    att = rng.random((CN, N), dtype=np.float32)
    W = (rng.standard_normal((OUT_F, IN_F), dtype=np.float32)
         / np.sqrt(IN_F)).astype(np.float32)
    dm = rng.integers(0, 2, size=(CN, N)).astype(bool)
    out = kernel(x=x, attention=att, W=W, drop_mask=dm)
    print("kernel out", out.shape, out.dtype, float(np.abs(out).max()))


# revision 47
# speedup vs baseline: 2.6404x; 1.0362x over previous
"""Trainium2 Bass kernel for GraphSpectralFilterLayer.

Reference computation:
    h = x @ W.T                                  [4096, 128]
    mean = attention.mean()                      (global scalar)
    a = where(att > mean, att, -9e15); LeakyReLU(0.2); softmax(dim=0)
    a = where(drop_mask, a / 0.4, 0)
    out = (a @ h) reshaped (4,4096,128)->(4096, 512)

Exact simplifications (att ~ U[0,1), mean > 0): kept values are positive
so LeakyReLU is identity; dropped values give exp(0.2*-9e15) == 0. Hence
v = exp(att) * (att > mean), softmax = v / colsum(v) (denominator does
NOT include the dropout mask). 1/(0.4*colsum) is folded into the tiny h.

Design (single pass over attention instead of three):
  - Phase 1 streams each att tile once: ACT caches t = att - 0.5 in a
    16 MiB bf16 SBUF cache while DVE accumulates mean partials via
    scalar_tensor_tensor accum_out. Caching the shifted t (not exp) puts
    full bf16 absolute precision at the threshold, so the phase-2 gate
    (t > mean - 0.5) is bit-faithful where it matters; the uniform
    e^0.5 factor cancels between numerator and colsum.
  - Mean finalized with a ones-matmul partition reduce + AllGather +
    local sum (flat AllGather is ~1.9x cheaper than AllReduce here).
  - Phase 2 per tile: ACT computes exp(t) into a small ring; one DVE
    scalar_tensor_tensor produces v = (t > m')*exp(t) in place in the
    cache WITH its column-sum (accum_out); vm = v * mask runs on GpSimd
    for the first 20 tiles (bf16 1/0 mask, host-converted) and DVE for
    the last 12 after its gate stream ends, so the gate stream is never
    interrupted. Colsums AllGather in 3 chunks (10/10/12, boundaries
    aligned with the engine split) with bounce/return DMAs routed via
    the SP and ACT DGEs to dodge the mask-ring-throttled queue; h
    scaling (ACT, per-partition reciprocal colsum) and the bf16 PSUM
    matmuls out[f,i] += h_s[jt].T @ vm[jt] chase each gather.
  - x/W are host-cast to bf16; h = x @ W.T on PE overlaps the mean
    AllGather window, pre-scaled by 1/0.4.

Sharding: rows of the [16384, 4096] attention matrix across 8 cores
(2048 rows each); attention/mask host-transposed so the contraction dim
(j, graph nodes) lands on partitions. softmax(dim=0) needs only the
[128, 32] colsum exchange + a scalar mean exchange. Output is produced
transposed [128, 2048] per core and un-transposed on host.
"""

import sys

sys.path.insert(0, "/opt/trn_rl_repo")

import os

import numpy as np

from concourse import bass, bacc, tile, mybir
from concourse.bass_utils import run_bass_kernel_spmd

N = 4096          # graph nodes (columns of attention)
CN = 16384        # C * N rows of attention
OUT_F = 128
IN_F = 512
N_CORES = 8
ROWS = CN // N_CORES          # 2048 local attention rows (i)
JT = N // 128                 # 32 j-tiles
NCN = float(CN) * float(N)    # mean divisor

F32 = mybir.dt.float32
BF16 = mybir.dt.bfloat16
U8 = mybir.dt.uint8
U16 = mybir.dt.uint16
AX = mybir.AxisListType
OP = mybir.AluOpType
AF = mybir.ActivationFunctionType



STAGE = int(os.environ.get("K_STAGE", "9"))
CHUNKS = [(0, 10), (10, 20), (20, 32)]
# gate stt runs on DVE only (walrus rejects TensorScalarPtr on Pool);
# the vm = v*mask tensor_tensor ops split: Pool takes the first 20
# tiles (ready earliest), DVE the last 12 after its stt stream.
POOL_TT = frozenset(range(20))
TT_ORDER = list(range(JT))


def _build():
    nc = bacc.Bacc("TRN2", target_bir_lowering=False, debug=False,
                   num_devices=N_CORES)

    attT = nc.dram_tensor("attT", [N, ROWS], F32, kind="ExternalInput")
    maskT = nc.dram_tensor("maskT", [N, ROWS], BF16, kind="ExternalInput")
    xT = nc.dram_tensor("xT", [IN_F, N], BF16, kind="ExternalInput")
    wT = nc.dram_tensor("wT", [IN_F, OUT_F], BF16, kind="ExternalInput")
    outT = nc.dram_tensor("outT", [OUT_F, ROWS], F32, kind="ExternalOutput")

    with tile.TileContext(nc) as tc:
        with tc.tile_pool(name="persist", bufs=1) as persist, \
             tc.tile_pool(name="mring", bufs=4) as mring, \
             tc.tile_pool(name="dram", bufs=1, space="DRAM") as dram:
            # ---- persistent SBUF state ----
            ecache = persist.tile([128, JT * ROWS], BF16)  # e, then vm
            h = persist.tile([128, JT * OUT_F], BF16)      # h per j-tile
            hs = persist.tile([128, JT * OUT_F], BF16)     # scaled h
            acc = persist.tile([128, JT], F32)             # mean partials
            cs = []
            for k, w in enumerate((10, 10, 12)):
                cs_k = persist.tile([128, w], F32, tag=f"cs{k}")
                cs.append(cs_k)
            csg = []
            for k in range(3):
                csg_k = persist.tile([128, N_CORES * 12], F32, tag=f"csg{k}")
                csg.append(csg_k)
            meang = persist.tile([128, N_CORES], F32)      # gathered mean
            ones_ff = persist.tile([128, 128], F32)
            tot = persist.tile([128, 1], F32)
            tot_bc = persist.tile([128, 1], F32)
            em = persist.tile([128, 1], F32)               # exp(mean)
            csum = persist.tile([128, JT], F32)
            rcs = persist.tile([128, JT], F32)
            nc.vector.memset(ones_ff[:, :], 1.0)

            # collective bounce buffers (DRAM, non-I/O)
            ag_mean_in = dram.tile([128, 1], F32)
            ag_mean_out = dram.tile([N_CORES, 128, 1], F32)
            ag_cs_in = []
            ag_cs_out = []
            for k, w in enumerate(c[1] - c[0] for c in CHUNKS):
                ag_cs_in_k = dram.tile([128, w], F32, tag=f"agci{k}")
                ag_cs_out_k = dram.tile([N_CORES, 128, w], F32,
                                        tag=f"agco{k}")
                ag_cs_in.append(ag_cs_in_k)
                ag_cs_out.append(ag_cs_out_k)

            if STAGE < 2:
                nc.compile()
                return nc
            # ---- Phase 1: stream att once; e=exp(att)->cache; mean ----
            m_ts = {}
            ctx2 = tc.tile_pool(name="vscr", bufs=3)
            vscr = ctx2.__enter__()
            # warm the Exp table early so phase 2's first exp doesn't pay
            # the table load on the critical path
            nc.scalar.activation(tot_bc[:, :], ones_ff[:, 0:1], AF.Exp)
            with tc.tile_pool(name="p1", bufs=3) as p1:
                for jt in range(JT):
                    a_t = p1.tile([128, ROWS], F32, tag="a1")
                    nc.sync.dma_start(out=a_t[:, :],
                                      in_=attT[jt * 128:(jt + 1) * 128, :])
                    nc.scalar.activation(
                        ecache[:, jt * ROWS:(jt + 1) * ROWS], a_t[:, :],
                        AF.Copy, bias=-0.5)
                    scr = vscr.tile([128, ROWS], BF16, tag="v")
                    nc.vector.scalar_tensor_tensor(
                        out=scr[:, :], in0=a_t[:, :], scalar=-1e30,
                        in1=a_t[:, :], op0=OP.is_gt, op1=OP.mult,
                        accum_out=acc[:, jt:jt + 1])

            def mask_fetch(jt):
                if jt in m_ts or jt >= JT:
                    return
                m_t = mring.tile([128, ROWS], BF16, tag="m")
                nc.sync.dma_start(out=m_t[:, :],
                                  in_=maskT[jt * 128:(jt + 1) * 128, :])
                m_ts[jt] = m_t

            # ---- mean: partition-reduce, AllGather, local sum ----
            with tc.tile_pool(name="mps", bufs=1, space="PSUM") as mps:
                nc.vector.tensor_reduce(out=tot[:, :], in_=acc[:, :],
                                        axis=AX.X, op=OP.add)
                ps_bc = mps.tile([128, 1], F32, tag="bc")
                nc.tensor.matmul(ps_bc[:, :], lhsT=ones_ff[:, :],
                                 rhs=tot[:, :], start=True, stop=True)
                nc.scalar.copy(tot_bc[:, :], ps_bc[:, :])
            nc.sync.dma_start(out=ag_mean_in[:, :], in_=tot_bc[:, :])
            nc.gpsimd.collective_compute(
                "AllGather", OP.bypass,
                replica_groups=[list(range(N_CORES))],
                ins=[ag_mean_in[:, :].opt()],
                outs=[ag_mean_out[:, :, :].opt()])
            # prefetch mask tiles in phase-2 consumption order; fills the
            # mean-AllGather window (issued after the mean bounce so they
            # don't delay it)
            for jt in TT_ORDER[:4]:
                mask_fetch(jt)
            nc.sync.dma_start(
                out=meang[:, 0:N_CORES],
                in_=ag_mean_out[:, :, :].rearrange("c p j -> p c j"))
            # tree-sum the 8 gathered copies -> total; em = exp(mean)
            nc.vector.tensor_tensor(out=meang[:, 0:4], in0=meang[:, 0:4],
                                    in1=meang[:, 4:8], op=OP.add)
            nc.vector.tensor_tensor(out=meang[:, 0:2], in0=meang[:, 0:2],
                                    in1=meang[:, 2:4], op=OP.add)
            nc.vector.tensor_tensor(out=meang[:, 0:1], in0=meang[:, 0:1],
                                    in1=meang[:, 1:2], op=OP.add)
            nc.scalar.activation(em[:, :], meang[:, 0:1], AF.Copy,
                                 scale=1.0 / NCN, bias=-0.5)

            # ---- h = x @ W.T  (bf16, from host-transposed xT, wT) ----
            with tc.tile_pool(name="xw", bufs=1) as xw, \
                 tc.tile_pool(name="hps", bufs=2, space="PSUM") as hps:
                wt_t = xw.tile([128, 4 * OUT_F], BF16, tag="wt")
                xt_ts = []
                for ct in range(4):
                    nc.sync.dma_start(
                        out=wt_t[:, ct * OUT_F:(ct + 1) * OUT_F],
                        in_=wT[ct * 128:(ct + 1) * 128, :])
                    xt_t = xw.tile([128, N], BF16, tag=f"xt{ct}")
                    nc.sync.dma_start(out=xt_t[:, :],
                                      in_=xT[ct * 128:(ct + 1) * 128, :])
                    xt_ts.append(xt_t)
                for jt in range(JT):
                    ps = hps.tile([128, OUT_F], F32, tag="hps")
                    for ct in range(4):
                        nc.tensor.matmul(
                            ps[:, :],
                            lhsT=xt_ts[ct][:, jt * 128:(jt + 1) * 128],
                            rhs=wt_t[:, ct * OUT_F:(ct + 1) * OUT_F],
                            start=(ct == 0), stop=(ct == 3))
                    nc.scalar.mul(h[:, jt * OUT_F:(jt + 1) * OUT_F],
                                  ps[:, :], 1.0 / (1.0 - 0.6))



            if STAGE < 3:
                ctx2.__exit__(None, None, None)
                nc.compile()
                return nc

            if STAGE < 4:
                ctx2.__exit__(None, None, None)
                nc.compile()
                return nc
            # ---- Phase 2 + chunked colsum AllGather + matmul ----
            # Streams are laid out so the Pool SEQ meets each collective
            # only after that collective's input is (nearly) ready -- a
            # collective's sem wait head-of-line-blocks everything behind
            # it on the Pool queue.  Chunk 1 is DVE-only so its colsum is
            # ready ~20us into phase 2; matmuls for each chunk run as soon
            # as its scaled h lands, overlapping the remaining gating.
            with tc.tile_pool(name="ops", bufs=1, space="PSUM") as ops:
                ps_o = []
                for ic in range(4):
                    ps_oc = ops.tile([128, 512], F32, tag=f"o{ic}")
                    ps_o.append(ps_oc)

                def e_slice(jt):
                    return ecache[:, jt * ROWS:(jt + 1) * ROWS]

                def cs_slot(jt):
                    for k, (lo, hi) in enumerate(CHUNKS):
                        if lo <= jt < hi:
                            return cs[k][:, jt - lo:jt - lo + 1]

                def stt(jt):
                    e_sl = e_slice(jt)
                    et = vscr.tile([128, ROWS], BF16, tag="v")
                    nc.scalar.activation(et[:, :], e_sl, AF.Exp)
                    nc.vector.scalar_tensor_tensor(
                        out=e_sl, in0=e_sl, scalar=em[:, 0:1],
                        in1=et[:, :], op0=OP.is_gt, op1=OP.mult,
                        accum_out=cs_slot(jt))

                def tt(jt):
                    if jt + 4 < JT:
                        mask_fetch(jt + 4)
                    e_sl = e_slice(jt)
                    eng = nc.gpsimd if jt in POOL_TT else nc.vector
                    eng.tensor_tensor(out=e_sl, in0=e_sl,
                                      in1=m_ts[jt][:, :], op=OP.mult)

                def cs_bounce(k):
                    lo, hi = CHUNKS[k]
                    nc.sync.dma_start(out=ag_cs_in[k][:, :], in_=cs[:, lo:hi])

                def cs_collective(k):
                    nc.gpsimd.collective_compute(
                        "AllGather", OP.bypass,
                        replica_groups=[list(range(N_CORES))],
                        ins=[ag_cs_in[k][:, :].opt()],
                        outs=[ag_cs_out[k][:, :, :].opt()])

                def cs_returns(k):
                    lo, hi = CHUNKS[k]
                    half = hi - lo
                    nc.scalar.dma_start(
                        out=csg[k][:, 0:N_CORES * half],
                        in_=ag_cs_out[k][:, :, :].rearrange("c p j -> p c j"))

                def cs_trees(k):
                    lo, hi = CHUNKS[k]
                    half = hi - lo
                    sl = slice(lo, hi)
                    nc.vector.tensor_tensor(
                        out=csg[k][:, 0:4 * half], in0=csg[k][:, 0:4 * half],
                        in1=csg[k][:, 4 * half:8 * half], op=OP.add)
                    nc.vector.tensor_tensor(
                        out=csg[k][:, 0:2 * half], in0=csg[k][:, 0:2 * half],
                        in1=csg[k][:, 2 * half:4 * half], op=OP.add)
                    nc.vector.tensor_tensor(
                        out=csum[:, sl], in0=csg[k][:, 0:half],
                        in1=csg[k][:, half:2 * half], op=OP.add)
                    nc.vector.reciprocal(rcs[:, sl], csum[:, sl])

                def scale_and_matmul(jts, last=False):
                    for jt in jts:
                        nc.scalar.mul(
                            hs[:, jt * OUT_F:(jt + 1) * OUT_F],
                            h[:, jt * OUT_F:(jt + 1) * OUT_F],
                            rcs[:, jt:jt + 1])
                        for ic in range(4):
                            nc.tensor.matmul(
                                ps_o[ic][:, :],
                                lhsT=hs[:, jt * OUT_F:(jt + 1) * OUT_F],
                                rhs=ecache[:, jt * ROWS + ic * 512:
                                           jt * ROWS + (ic + 1) * 512],
                                start=(jt == 0), stop=(jt == JT - 1))

                C1, C2, C3 = [list(range(*c)) for c in CHUNKS]

                for jt in C1:
                    stt(jt)
                cs_bounce(0)
                cs_collective(0)
                for jt in C1:
                    tt(jt)            # pool
                for jt in C2:
                    stt(jt)
                cs_bounce(1)
                cs_collective(1)
                for jt in C2:
                    tt(jt)            # pool / dve tail
                cs_finish(0)
                scale_and_matmul(C1)
                for jt in C3:
                    stt(jt)
                cs_bounce(2)
                cs_collective(2)
                for jt in C3:
                    tt(jt)            # dve
                cs_finish(1)
                scale_and_matmul(C2)
                cs_finish(2)
                scale_and_matmul(C3)
                ctx2.__exit__(None, None, None)
                # ---- drain PSUM -> SBUF -> HBM ----
                with tc.tile_pool(name="outp", bufs=2) as outp:
                    for ic in range(4):
                        o_t = outp.tile([128, 512], F32, tag="osb")
                        nc.scalar.copy(o_t[:, :], ps_o[ic][:, :])
                        nc.sync.dma_start(
                            out=outT[:, ic * 512:(ic + 1) * 512],
                            in_=o_t[:, :])
    nc.compile()
    return nc


def kernel(x, attention, W, drop_mask):
    import ml_dtypes
    attT = np.ascontiguousarray(attention.T)           # [4096, 16384] f32
    maskT = np.ascontiguousarray(
        drop_mask.T.astype(ml_dtypes.bfloat16))        # 1.0 / 0.0
    xTb = np.ascontiguousarray(x.T).astype(ml_dtypes.bfloat16)  # [512, 4096]
    wTb = np.ascontiguousarray(W.T).astype(ml_dtypes.bfloat16)  # [512, 128]

    nc = _build()
    in_maps = []
    for c in range(N_CORES):
        sl = slice(c * ROWS, (c + 1) * ROWS)
        in_maps.append({
            "attT": np.ascontiguousarray(attT[:, sl]),
            "maskT": np.ascontiguousarray(maskT[:, sl]),
            "xT": xTb,
            "wT": wTb,
        })
    res = run_bass_kernel_spmd(nc, in_maps, core_ids=list(range(N_CORES)))
    global LAST_EXEC_NS
    LAST_EXEC_NS = res.exec_time_ns or res.mean_exec_time_ns
    h_prime = np.concatenate(
        [res.results[c]["outT"].T for c in range(N_CORES)], axis=0)
    out = (h_prime.reshape(4, N, OUT_F).transpose(1, 0, 2)
           .reshape(N, 4 * OUT_F))
    return np.ascontiguousarray(out)


if __name__ == "__main__":
    rng = np.random.default_rng(0)
    x = rng.standard_normal((N, IN_F), dtype=np.float32)
    att = rng.random((CN, N), dtype=np.float32)
    W = (rng.standard_normal((OUT_F, IN_F), dtype=np.float32)
         / np.sqrt(IN_F)).astype(np.float32)
    dm = rng.integers(0, 2, size=(CN, N)).astype(bool)
    out = kernel(x=x, attention=att, W=W, drop_mask=dm)
    print("kernel out", out.shape, out.dtype, float(np.abs(out).max()))
